# revision 8
# baseline (speedup 1.0000x reference)
"""NodeRoIPool Trainium2 kernel (v4).

For each of 20000 ROIs (8 corner coords), 5 points (4 edge midpoints +
centroid) are snapped to the feature grid (ceil, clip to [2,254]) and a
4x4 window of feat [256,256,256] is mean-pooled across all 256 channels,
giving out [20000, 1280] (point-major, channel-fastest).

Algorithm: the 4x4 sum only depends on the snapped point, so precompute
a 4x4 box-filtered map once, kept CHANNEL-MAJOR and SBUF-RESIDENT as
boxsb[c, y, x] bf16 (16 MB); each point then becomes a single gpsimd
ap_gather of the x-pair (y*128 + x//2) along the free dimension -- no
transposes and no DRAM round trip. Both pair halves are written out;
the host mirrors the f32 snap arithmetic (bit-exact) to select the
correct half, applies the 1/16 mean scale, and reassembles the output.

Sharding (8 cores): 2-way channel x 4-way ROI.
"""

import numpy as np

import concourse.bass as bass
import concourse.tile as tile
from concourse import bacc, mybir
from concourse import bass_utils

N_CORES = 8
CH_SHARD = 2          # channel shards (128 ch per core)
ROI_SHARD = 4         # ROI shards (5000 rois per core)
C, H, W = 256, 256, 256
CS = C // CH_SHARD    # 128 channels per core
N_ROIS = 20000
RPC = N_ROIS // ROI_SHARD          # 5000 rois per core
RP_PAD = 5120                       # padded to 40 rois per partition
RPP = RP_PAD // 128                 # 40 rois per partition
G = RPP * 5                         # 200 points per partition
NPTS_PAD = 128 * G                  # 25600 gathered points per core
YCHUNK = 16                         # output rows of the box filter per chunk
GCALLS = 8                          # gather calls
GN = NPTS_PAD // GCALLS             # 3200 points per gather call
NPAIRS = H * W // 2                 # 32768 x-pairs in the box map
F32 = mybir.dt.float32
BF16 = mybir.dt.bfloat16
I32 = mybir.dt.int32
I16 = mybir.dt.int16

_prog_cache = {}


def _build_program():
    nc = bacc.Bacc("TRN2", target_bir_lowering=False, debug=False,
                   num_devices=N_CORES, dynamic_dma_scratch_size=2048)

    feat_in = nc.dram_tensor("feat", [CS, H, W], F32, kind="ExternalInput")
    rois_in = nc.dram_tensor("rois", [RP_PAD, 8], F32, kind="ExternalInput")
    out_t = nc.dram_tensor("out", [128, NPTS_PAD, 2], BF16,
                           kind="ExternalOutput")

    with tile.TileContext(nc) as tc:
        with tc.tile_pool(name="sbuf", bufs=1) as pool:
            # persistent: the SBUF-resident box-filtered map (128 KiB/part)
            boxsb = pool.tile([128, H, W], BF16, tag="boxsb")
            idx16 = pool.tile([128, G], I16, tag="idx16")
            idx_w = []
            for c in range(GCALLS):
                w = pool.tile([128, G], I16, tag=f"idxw{c}")
                idx_w.append(w)

            # ---------------- point indices from rois -------------------
            with tc.tile_pool(name="idxp", bufs=1) as xp:
                roi_t = xp.tile([128, RPP, 8], F32, tag="roi")
                nc.sync.dma_start(
                    out=roi_t[:],
                    in_=rois_in.rearrange("(p r) c -> p r c", p=128),
                )
                rr = xp.tile([128, RPP, 8], F32, tag="rr")
                nc.vector.tensor_scalar_mul(rr[:], roi_t[:], 0.25)

                # points [128, RPP, 5] per coordinate, point k = slot k
                idx_f = {}
                for d in range(2):  # 0=x, 1=y
                    pts = xp.tile([128, RPP, 5], F32, tag=f"pts{d}")
                    # mids k=0..2: rr[2k+d] + rr[2k+2+d]
                    nc.vector.tensor_tensor(
                        out=pts[:, :, 0:3],
                        in0=rr[:, :, d:d + 5:2],
                        in1=rr[:, :, d + 2:d + 7:2],
                        op=mybir.AluOpType.add,
                    )
                    # mid k=3 wraps: rr[6+d] + rr[d]
                    nc.vector.tensor_tensor(
                        out=pts[:, :, 3:4],
                        in0=rr[:, :, d + 6:d + 7],
                        in1=rr[:, :, d:d + 1],
                        op=mybir.AluOpType.add,
                    )
                    nc.vector.tensor_scalar_mul(
                        pts[:, :, 0:4], pts[:, :, 0:4], 0.5)
                    # centroid, sequential sum order ((c0+c1)+c2)+c3
                    nc.vector.tensor_tensor(
                        out=pts[:, :, 4:5], in0=rr[:, :, d:d + 1],
                        in1=rr[:, :, d + 2:d + 3], op=mybir.AluOpType.add)
                    nc.vector.tensor_tensor(
                        out=pts[:, :, 4:5], in0=pts[:, :, 4:5],
                        in1=rr[:, :, d + 4:d + 5], op=mybir.AluOpType.add)
                    nc.vector.tensor_tensor(
                        out=pts[:, :, 4:5], in0=pts[:, :, 4:5],
                        in1=rr[:, :, d + 6:d + 7], op=mybir.AluOpType.add)
                    nc.vector.tensor_scalar_mul(
                        pts[:, :, 4:5], pts[:, :, 4:5], 0.25)

                    # ceil(x) = n + (x > n) where n = int-cast(x); works for
                    # either truncating or round-to-nearest casts.
                    ni = xp.tile([128, RPP, 5], I32, tag=f"ni{d}")
                    nc.vector.tensor_copy(out=ni[:], in_=pts[:])
                    tt = xp.tile([128, RPP, 5], F32, tag=f"tt{d}")
                    nc.vector.tensor_copy(out=tt[:], in_=ni[:])
                    gt = xp.tile([128, RPP, 5], F32, tag=f"gt{d}")
                    nc.vector.tensor_tensor(
                        out=gt[:], in0=pts[:], in1=tt[:],
                        op=mybir.AluOpType.is_gt)
                    nc.vector.tensor_tensor(
                        out=tt[:], in0=tt[:], in1=gt[:],
                        op=mybir.AluOpType.add)
                    # clip to [2, 254]
                    nc.vector.tensor_scalar(
                        out=tt[:], in0=tt[:], scalar1=2.0, scalar2=254.0,
                        op0=mybir.AluOpType.max, op1=mybir.AluOpType.min)
                    idx_f[d] = tt

                # pair index = y*128 + floor(x/2) <= 32639, int16-safe.
                xcf, ycf = idx_f[0], idx_f[1]
                xh = xp.tile([128, RPP, 5], F32, tag="xh")
                nc.vector.tensor_scalar_mul(xh[:], xcf[:], 0.5)
                # floor(t) = n - (n > t) for either cast rounding mode
                ni2 = xp.tile([128, RPP, 5], I32, tag="ni2")
                nc.vector.tensor_copy(out=ni2[:], in_=xh[:])
                fl = xp.tile([128, RPP, 5], F32, tag="fl")
                nc.vector.tensor_copy(out=fl[:], in_=ni2[:])
                gt2 = xp.tile([128, RPP, 5], F32, tag="gt2")
                nc.vector.tensor_tensor(
                    out=gt2[:], in0=fl[:], in1=xh[:], op=mybir.AluOpType.is_gt)
                nc.vector.tensor_tensor(
                    out=fl[:], in0=fl[:], in1=gt2[:],
                    op=mybir.AluOpType.subtract)
                # flat = y*128 + floor(x/2)
                flat_f = xp.tile([128, RPP, 5], F32, tag="flatf")
                nc.vector.scalar_tensor_tensor(
                    out=flat_f[:], in0=ycf[:], scalar=128.0, in1=fl[:],
                    op0=mybir.AluOpType.mult, op1=mybir.AluOpType.add)
                nc.vector.tensor_copy(
                    out=idx16[:].rearrange("p (r k) -> p r k", k=5),
                    in_=flat_f[:])

                # ap_gather reads indices from each group of 16 partitions
                # (slot s, part q -> stream position i = s*16+q); replicate
                # window [16c, 16c+16) to all 8 groups. Stream i of call c
                # is point (16c + i%16)*G + i//16; the host inverts that.
                engs = [nc.sync, nc.scalar, nc.sync, nc.scalar]
                for c in range(GCALLS):
                    for u in range(8):
                        engs[u % 4].dma_start(
                            out=idx_w[c][16 * u:16 * u + 16, :],
                            in_=idx16[16 * c:16 * c + 16, :])

            # never-snapped rows/cols (y,x in {0,1,255}): zero so pair
            # gathers stay finite
            nc.vector.memset(boxsb[:, 0:2, :], 0.0)
            nc.vector.memset(boxsb[:, H - 1:H, :], 0.0)
            nc.vector.memset(boxsb[:, :, 0:2], 0.0)
            nc.vector.memset(boxsb[:, :, W - 1:W], 0.0)

            # ---------------- box filter ---------------------------------
            # 4x4 box SUM (the /16 is applied host-side) with windows
            # [i-2, i+1] in both axes, written straight into boxsb.
            with (
                tc.tile_pool(name="io", bufs=2) as iop,
                tc.tile_pool(name="flt", bufs=1) as fp,
            ):
                for ci in range(H // YCHUNK):
                    a = max(2, ci * YCHUNK)            # first valid out row
                    b = min(H - 1, (ci + 1) * YCHUNK)  # end of valid rows
                    nv = b - a
                    ys0 = a - 2
                    ys1 = min(H, b + 1)                # u[y] needs h[y+1]
                    nr = ys1 - ys0                     # loaded rows (<= 19)

                    fin = iop.tile([128, YCHUNK + 3, W], F32, tag="fin")
                    (nc.scalar if ci % 2 else nc.sync).dma_start(
                        out=fin[:, 0:nr, :], in_=feat_in[:, ys0:ys1, :])

                    # first add reads f32 (full DVE f32 rate), emits bf16
                    s1 = fp.tile([128, YCHUNK + 3, W - 1], BF16, tag="s1")
                    nc.vector.tensor_tensor(
                        out=s1[:, 0:nr, :], in0=fin[:, 0:nr, 0:W - 1],
                        in1=fin[:, 0:nr, 1:W], op=mybir.AluOpType.add)
                    hh = fp.tile([128, YCHUNK + 3, W], BF16, tag="hh")
                    nc.vector.tensor_tensor(
                        out=hh[:, 0:nr, 2:W - 1], in0=s1[:, 0:nr, 0:W - 3],
                        in1=s1[:, 0:nr, 2:W - 1], op=mybir.AluOpType.add)
                    uu = fp.tile([128, YCHUNK + 2, W], BF16, tag="uu")
                    nc.vector.tensor_tensor(
                        out=uu[:, 0:nr - 1, 2:W - 1],
                        in0=hh[:, 0:nr - 1, 2:W - 1],
                        in1=hh[:, 1:nr, 2:W - 1], op=mybir.AluOpType.add)
                    # v[y'] = u[y'-2] + u[y'], written directly into boxsb
                    o0 = a - 2 - ys0
                    o1 = a - ys0
                    nc.vector.tensor_tensor(
                        out=boxsb[:, a:b, 2:W - 1],
                        in0=uu[:, o0:o0 + nv, 2:W - 1],
                        in1=uu[:, o1:o1 + nv, 2:W - 1],
                        op=mybir.AluOpType.add)

            # ---------------- gather + writeback -------------------------
            # boxsb as x-pairs [c, 32768, 2]; out row i of call c is
            # stream position i (point (16c + i%16)*G + i//16).
            pairs = boxsb[:].rearrange("p y (t d) -> p (y t) d", d=2)
            with tc.tile_pool(name="go", bufs=2) as gop:
                for gi in range(GCALLS):
                    go = gop.tile([128, GN, 2], BF16, tag="go")
                    nc.gpsimd.ap_gather(
                        go[:],
                        pairs,
                        idx_w[gi][:],
                        channels=128,
                        num_elems=NPAIRS,
                        d=2,
                        num_idxs=GN,
                    )
                    nc.sync.dma_start(
                        out=out_t[:, gi * GN:(gi + 1) * GN, :],
                        in_=go[:])

    nc.compile()
    return nc


def _host_xpar(rois: np.ndarray) -> np.ndarray:
    """Mirror the device f32 snap math; return per-point x parity (0/1).

    Bit-exact with the DVE ops: same f32 operand order for mids/centroid,
    ceil == int-snap + adjust == np.ceil for x >= 0, then clip [2, 254].
    """
    rr = (rois.astype(np.float32) * np.float32(0.25)).reshape(-1, 4, 2)
    mids = (rr + np.roll(rr, -1, axis=1)) * np.float32(0.5)
    ctr = ((rr[:, 0] + rr[:, 1]) + rr[:, 2]) + rr[:, 3]
    ctr = ctr * np.float32(0.25)
    px = np.concatenate([mids[:, :, 0], ctr[:, 0:1]], axis=1)  # [N, 5]
    xc = np.clip(np.ceil(px), np.float32(2.0), np.float32(254.0))
    return (xc.astype(np.int64)) & 1


def kernel(feat: np.ndarray, rois: np.ndarray) -> np.ndarray:
    feat = np.ascontiguousarray(np.asarray(feat, dtype=np.float32))
    rois = np.ascontiguousarray(np.asarray(rois, dtype=np.float32))
    assert feat.shape == (C, H, W) and rois.shape == (N_ROIS, 8)

    if "nc" not in _prog_cache:
        _prog_cache["nc"] = _build_program()
    nc = _prog_cache["nc"]

    rois_pad = np.zeros((RP_PAD, 8), dtype=np.float32)
    in_maps = []
    for core in range(N_CORES):
        ci, ri = divmod(core, ROI_SHARD)
        rp = rois_pad.copy()
        rp[:RPC] = rois[ri * RPC:(ri + 1) * RPC]
        in_maps.append({
            "feat": np.ascontiguousarray(feat[ci * CS:(ci + 1) * CS]),
            "rois": rp,
        })

    res = bass_utils.run_bass_kernel_spmd(
        nc, in_maps, core_ids=list(range(N_CORES)))

    # out row i of call c holds point (16c + i%16)*G + i//16
    r = np.arange(NPTS_PAD)
    gc, i = divmod(r, GN)
    perm = (16 * gc + i % 16) * G + i // 16

    xpar = _host_xpar(rois)  # [N_ROIS, 5]
    out = np.empty((ROI_SHARD, RPC, 5, CH_SHARD, CS), dtype=np.float32)
    pts = np.empty((NPTS_PAD, 2, CS), dtype=np.float32)
    for core in range(N_CORES):
        ci, ri = divmod(core, ROI_SHARD)
        raw = np.asarray(res.results[core]["out"])  # [128, NPTS_PAD, 2] bf16
        # device values are unscaled 4x4 sums; apply the mean's 1/16 here
        pts[perm] = raw.transpose(1, 2, 0).astype(np.float32)
        pts *= np.float32(1.0 / 16.0)
        both = pts[:RPC * 5].reshape(RPC, 5, 2, CS)
        sel = np.take_along_axis(
            both,
            xpar[ri * RPC:(ri + 1) * RPC][:, :, None, None],
            axis=2,
        )[:, :, 0]
        out[ri, :, :, ci, :] = sel
    return out.reshape(N_ROIS, 5 * C)


# revision 12
# speedup vs baseline: 1.4103x; 1.4103x over previous
"""NodeRoIPool Trainium2 kernel (v5b).

For each of 20000 ROIs (8 corner coords), 5 points (4 edge midpoints +
centroid) are snapped to the feature grid (ceil, clip to [2,254]) and a
4x4 window of feat [256,256,256] is mean-pooled across all 256 channels,
giving out [20000, 1280] (point-major, channel-fastest).

Algorithm: the 4x4 sum only depends on the snapped point, so precompute
a 4x4 box-filtered map once in bf16, stored channel-last in DRAM as
boxfeat[row, c] with row = xl*512 + y*2 + xb (x = xb*128 + xl); each
point then becomes a single 512 B row-pair dma_gather (pair index
xl*256 + y <= 32766, int16).

Per core (2-way channel x 4-way ROI sharding):
  - filter: scalar-ACT converts feat chunks to bf16, DVE does the four
    separable shift-adds (SUM only; the /16 mean scale is applied
    host-side), one XBAR dma_start_transpose per y-chunk rearranges
    [c, (y,x)] -> [xl, (y,xb), c], contiguous 8 KiB-run DMA to DRAM.
  - 25600 pair indices computed on-device from the rois.
  - 8 dma_gather calls spread over 4 SWDGE queues fetch the row pairs;
    both x-halves go to out [25600, 256] bf16.
The host mirrors the f32 snap arithmetic (bit-exact) to pick the
correct x-half per point and reassembles the [20000, 1280] f32 output.
"""

import numpy as np

import concourse.bass as bass
import concourse.tile as tile
from concourse import bacc, mybir
from concourse import bass_utils

N_CORES = 8
CH_SHARD = 2          # channel shards (128 ch per core)
ROI_SHARD = 4         # ROI shards (5000 rois per core)
C, H, W = 256, 256, 256
CS = C // CH_SHARD    # 128 channels per core
N_ROIS = 20000
RPC = N_ROIS // ROI_SHARD          # 5000 rois per core
RP_PAD = 5120                       # padded to 40 rois per partition
RPP = RP_PAD // 128                 # 40 rois per partition
G = RPP * 5                         # 200 points per partition
NPTS_PAD = 128 * G                  # 25600 gathered points per core
YCHUNK = 16                         # output rows of the box filter per chunk
GCALLS = 8                          # gather calls
GN = NPTS_PAD // GCALLS             # 3200 points per gather call
GSL = GN // 128                     # 25 out slots per partition per call
F32 = mybir.dt.float32
BF16 = mybir.dt.bfloat16
I32 = mybir.dt.int32
I16 = mybir.dt.int16

_prog_cache = {}


def _build_program():
    nc = bacc.Bacc("TRN2", target_bir_lowering=False, debug=False,
                   num_devices=N_CORES, num_swdge_queues=4)

    feat_in = nc.dram_tensor("feat", [CS, H, W], F32, kind="ExternalInput")
    rois_in = nc.dram_tensor("rois", [RP_PAD, 8], F32, kind="ExternalInput")
    out_t = nc.dram_tensor("out", [NPTS_PAD, 2 * CS], BF16,
                           kind="ExternalOutput")
    # row = xl*512 + y*2 + xb  (x = xb*128 + xl), c contiguous
    boxfeat = nc.dram_tensor("boxfeat", [H * W, CS], BF16, kind="Internal")

    with tile.TileContext(nc) as tc:
        with (
            tc.tile_pool(name="sbuf", bufs=1) as pool,
            tc.tile_pool(name="io", bufs=2) as iop,
        ):
            idx16 = pool.tile([128, G], I16, tag="idx16")
            idx_w = []
            for c in range(GCALLS):
                w = pool.tile([128, G], I16, tag=f"idxw{c}")
                idx_w.append(w)

            # ---------------- point indices from rois -------------------
            if True:
                xp = pool
                roi_t = xp.tile([128, RPP, 8], F32, tag="roi")
                nc.sync.dma_start(
                    out=roi_t[:],
                    in_=rois_in.rearrange("(p r) c -> p r c", p=128),
                )
                rr = xp.tile([128, RPP, 8], F32, tag="rr")
                nc.vector.tensor_scalar_mul(rr[:], roi_t[:], 0.25)

                # points [128, RPP, 5] per coordinate, point k = slot k
                idx_f = {}
                for d in range(2):  # 0=x, 1=y
                    pts = xp.tile([128, RPP, 5], F32, tag=f"pts{d}")
                    # mids k=0..2: rr[2k+d] + rr[2k+2+d]
                    nc.vector.tensor_tensor(
                        out=pts[:, :, 0:3],
                        in0=rr[:, :, d:d + 5:2],
                        in1=rr[:, :, d + 2:d + 7:2],
                        op=mybir.AluOpType.add,
                    )
                    # mid k=3 wraps: rr[6+d] + rr[d]
                    nc.vector.tensor_tensor(
                        out=pts[:, :, 3:4],
                        in0=rr[:, :, d + 6:d + 7],
                        in1=rr[:, :, d:d + 1],
                        op=mybir.AluOpType.add,
                    )
                    nc.vector.tensor_scalar_mul(
                        pts[:, :, 0:4], pts[:, :, 0:4], 0.5)
                    # centroid, sequential sum order ((c0+c1)+c2)+c3
                    nc.vector.tensor_tensor(
                        out=pts[:, :, 4:5], in0=rr[:, :, d:d + 1],
                        in1=rr[:, :, d + 2:d + 3], op=mybir.AluOpType.add)
                    nc.vector.tensor_tensor(
                        out=pts[:, :, 4:5], in0=pts[:, :, 4:5],
                        in1=rr[:, :, d + 4:d + 5], op=mybir.AluOpType.add)
                    nc.vector.tensor_tensor(
                        out=pts[:, :, 4:5], in0=pts[:, :, 4:5],
                        in1=rr[:, :, d + 6:d + 7], op=mybir.AluOpType.add)
                    nc.vector.tensor_scalar_mul(
                        pts[:, :, 4:5], pts[:, :, 4:5], 0.25)

                    # ceil(x) = n + (x > n) where n = int-cast(x); works for
                    # either truncating or round-to-nearest casts.
                    ni = xp.tile([128, RPP, 5], I32, tag=f"ni{d}")
                    nc.vector.tensor_copy(out=ni[:], in_=pts[:])
                    tt = xp.tile([128, RPP, 5], F32, tag=f"tt{d}")
                    nc.vector.tensor_copy(out=tt[:], in_=ni[:])
                    gt = xp.tile([128, RPP, 5], F32, tag=f"gt{d}")
                    nc.vector.tensor_tensor(
                        out=gt[:], in0=pts[:], in1=tt[:],
                        op=mybir.AluOpType.is_gt)
                    nc.vector.tensor_tensor(
                        out=tt[:], in0=tt[:], in1=gt[:],
                        op=mybir.AluOpType.add)
                    # clip to [2, 254]
                    nc.vector.tensor_scalar(
                        out=tt[:], in0=tt[:], scalar1=2.0, scalar2=254.0,
                        op0=mybir.AluOpType.max, op1=mybir.AluOpType.min)
                    idx_f[d] = tt

                # pair index = xl*256 + y with xl = x - 128*(x >= 128);
                # max 127*256 + 254 = 32766, int16-safe.
                xcf, ycf = idx_f[0], idx_f[1]
                xbf = xp.tile([128, RPP, 5], F32, tag="xbf")
                nc.vector.tensor_scalar(
                    out=xbf[:], in0=xcf[:], scalar1=128.0, scalar2=None,
                    op0=mybir.AluOpType.is_ge)
                xlf = xp.tile([128, RPP, 5], F32, tag="xlf")
                nc.vector.scalar_tensor_tensor(
                    out=xlf[:], in0=xbf[:], scalar=-128.0, in1=xcf[:],
                    op0=mybir.AluOpType.mult, op1=mybir.AluOpType.add)
                flat_f = xp.tile([128, RPP, 5], F32, tag="flatf")
                nc.vector.scalar_tensor_tensor(
                    out=flat_f[:], in0=xlf[:], scalar=256.0, in1=ycf[:],
                    op0=mybir.AluOpType.mult, op1=mybir.AluOpType.add)
                nc.vector.tensor_copy(
                    out=idx16[:].rearrange("p (r k) -> p r k", k=5),
                    in_=flat_f[:])

                # dma_gather reads indices from partitions 0..15 (slot s,
                # part q -> stream position i = s*16+q), replicated to all
                # 8 groups of 16 partitions. Gather call c uses window
                # [16c, 16c+16): stream i of call c is point
                # (16c + i%16)*G + i//16; the host inverts that.
                engs = [nc.sync, nc.scalar, nc.sync, nc.scalar]
                for c in range(GCALLS):
                    for u in range(8):
                        engs[u % 4].dma_start(
                            out=idx_w[c][16 * u:16 * u + 16, :],
                            in_=idx16[16 * c:16 * c + 16, :])

            # boxfeat viewed [xl, y, xb, c]
            dst0 = boxfeat.rearrange("(xl y xb) c -> xl y xb c", xl=128, xb=2)

            # rows with y in {0,1,255} are never snapped to but are read
            # as pair halves: zero-fill so every gather read is finite.
            zt = pool.tile([128, 3, 2, CS], BF16, tag="zt")
            nc.vector.memset(zt[:], 0.0)
            nc.sync.dma_start(out=dst0[:, 0:2, :, :], in_=zt[:, 0:2])
            nc.sync.dma_start(out=dst0[:, 255:256, :, :], in_=zt[:, 2:3])

            # ---------------- box filter ---------------------------------
            # 4x4 box SUM (the /16 is applied host-side) with windows
            # [i-2, i+1] in both axes.
            if True:
                fp = pool
                for ci in range(H // YCHUNK):
                    a = max(2, ci * YCHUNK)            # first valid out row
                    b = min(H - 1, (ci + 1) * YCHUNK)  # end of valid rows
                    nv = b - a
                    ys0 = a - 2
                    ys1 = min(H, b + 1)                # u[y] needs h[y+1]
                    nr = ys1 - ys0                     # loaded rows (<= 19)

                    fin = iop.tile([128, YCHUNK + 3, W], F32, tag="fin")
                    nc.scalar.dma_start(
                        out=fin[:, 0:nr, :], in_=feat_in[:, ys0:ys1, :])
                    # bf16 convert on the otherwise-idle scalar ACT engine
                    # so every DVE add runs at the 16-bit rate
                    fb = iop.tile([128, YCHUNK + 3, W], BF16, tag="fb")
                    nc.scalar.activation(
                        out=fb[:, 0:nr, :], in_=fin[:, 0:nr, :],
                        func=mybir.ActivationFunctionType.Copy, scale=1.0)

                    s1 = fp.tile([128, YCHUNK + 3, W - 1], BF16, tag="s1")
                    nc.vector.tensor_tensor(
                        out=s1[:, 0:nr, :], in0=fb[:, 0:nr, 0:W - 1],
                        in1=fb[:, 0:nr, 1:W], op=mybir.AluOpType.add)
                    hh = fp.tile([128, YCHUNK + 3, W], BF16, tag="hh")
                    nc.vector.tensor_tensor(
                        out=hh[:, 0:nr, 2:W - 1], in0=s1[:, 0:nr, 0:W - 3],
                        in1=s1[:, 0:nr, 2:W - 1], op=mybir.AluOpType.add)
                    uu = fp.tile([128, YCHUNK + 2, W], BF16, tag="uu")
                    nc.vector.tensor_tensor(
                        out=uu[:, 0:nr - 1, 2:W - 1],
                        in0=hh[:, 0:nr - 1, 2:W - 1],
                        in1=hh[:, 1:nr, 2:W - 1], op=mybir.AluOpType.add)
                    # v[y'] = u[y'-2] + u[y']; double-buffered (consumed by
                    # the XBAR on another engine)
                    vv = iop.tile([128, YCHUNK, W], BF16, tag="vv")
                    nc.vector.memset(vv[:, :, 0:2], 0.0)
                    nc.vector.memset(vv[:, :, W - 1:W], 0.0)
                    o0 = a - 2 - ys0
                    o1 = a - ys0
                    nc.vector.tensor_tensor(
                        out=vv[:, 0:nv, 2:W - 1],
                        in0=uu[:, o0:o0 + nv, 2:W - 1],
                        in1=uu[:, o1:o1 + nv, 2:W - 1],
                        op=mybir.AluOpType.add)

                    # XBAR: stg[xl, y*2+xb, c] = vv[c, y, xb*128+xl]
                    stg = iop.tile([128, YCHUNK, 2, CS], BF16, tag="stg")
                    # keep every XBAR on ONE queue: concurrent transposes on
                    # both hwdge queues corrupt data (shared crossbar)
                    nc.sync.dma_start_transpose(
                        out=stg[:, 0:nv, :, :],
                        in_=vv[:, 0:nv, :],
                    )
                    # per-partition contiguous runs of nv*2*CS bf16 (8 KiB)
                    nc.sync.dma_start(
                        out=dst0[:, a:b, :, :],
                        in_=stg[:, 0:nv, :, :],
                    )

            # ---------------- gather + writeback -------------------------
            # DRAM row r = gi*GN + stream i; host un-permutes to point order
            out_v = out_t.rearrange("(s p) c -> p s c", p=128)
            pairs = boxfeat.rearrange("(r two) c -> r (two c)", two=2)
            if True:
                gop = iop
                for gi in range(GCALLS):
                    gt = gop.tile([128, GSL, 2 * CS], BF16, tag="gt")
                    nc.gpsimd.dma_gather(
                        gt[:],
                        pairs,
                        idx_w[gi][:],
                        GN,
                        GN,
                        2 * CS,
                        single_packet=False,
                        queue_num=gi % 4,
                    )
                    nc.sync.dma_start(
                        out=out_v[:, gi * GSL:(gi + 1) * GSL, :],
                        in_=gt[:])

    nc.compile()
    return nc


def _host_xhalf(rois: np.ndarray) -> np.ndarray:
    """Mirror the device f32 snap math; return per-point x-half (0/1).

    Bit-exact with the DVE ops: same f32 operand order for mids/centroid,
    ceil == int-snap + adjust == np.ceil for x >= 0, then clip [2, 254].
    """
    rr = (rois.astype(np.float32) * np.float32(0.25)).reshape(-1, 4, 2)
    mids = (rr + np.roll(rr, -1, axis=1)) * np.float32(0.5)
    ctr = ((rr[:, 0] + rr[:, 1]) + rr[:, 2]) + rr[:, 3]
    ctr = ctr * np.float32(0.25)
    px = np.concatenate([mids[:, :, 0], ctr[:, 0:1]], axis=1)  # [N, 5]
    xc = np.clip(np.ceil(px), np.float32(2.0), np.float32(254.0))
    return (xc >= 128.0).astype(np.int64)


def kernel(feat: np.ndarray, rois: np.ndarray) -> np.ndarray:
    feat = np.ascontiguousarray(np.asarray(feat, dtype=np.float32))
    rois = np.ascontiguousarray(np.asarray(rois, dtype=np.float32))
    assert feat.shape == (C, H, W) and rois.shape == (N_ROIS, 8)

    if "nc" not in _prog_cache:
        _prog_cache["nc"] = _build_program()
    nc = _prog_cache["nc"]

    rois_pad = np.zeros((RP_PAD, 8), dtype=np.float32)
    in_maps = []
    for core in range(N_CORES):
        ci, ri = divmod(core, ROI_SHARD)
        rp = rois_pad.copy()
        rp[:RPC] = rois[ri * RPC:(ri + 1) * RPC]
        in_maps.append({
            "feat": np.ascontiguousarray(feat[ci * CS:(ci + 1) * CS]),
            "rois": rp,
        })

    res = bass_utils.run_bass_kernel_spmd(
        nc, in_maps, core_ids=list(range(N_CORES)))

    # DRAM row r = c*GN + i holds point (16c + i%16)*G + i//16
    r = np.arange(NPTS_PAD)
    gc, i = divmod(r, GN)
    perm = (16 * gc + i % 16) * G + i // 16

    xhalf = _host_xhalf(rois)  # [N_ROIS, 5]
    out = np.empty((ROI_SHARD, RPC, 5, CH_SHARD, CS), dtype=np.float32)
    pts = np.empty((NPTS_PAD, 2 * CS), dtype=np.float32)
    for core in range(N_CORES):
        ci, ri = divmod(core, ROI_SHARD)
        # device values are unscaled 4x4 sums; apply the mean's 1/16 here
        pts[perm] = np.asarray(res.results[core]["out"]).astype(np.float32)
        pts *= np.float32(1.0 / 16.0)
        both = pts[:RPC * 5].reshape(RPC, 5, 2, CS)
        sel = np.take_along_axis(
            both,
            xhalf[ri * RPC:(ri + 1) * RPC][:, :, None, None],
            axis=2,
        )[:, :, 0]
        out[ri, :, :, ci, :] = sel
    return out.reshape(N_ROIS, 5 * C)


# revision 16
# speedup vs baseline: 1.4506x; 1.0286x over previous
"""NodeRoIPool Trainium2 kernel (v5b).

For each of 20000 ROIs (8 corner coords), 5 points (4 edge midpoints +
centroid) are snapped to the feature grid (ceil, clip to [2,254]) and a
4x4 window of feat [256,256,256] is mean-pooled across all 256 channels,
giving out [20000, 1280] (point-major, channel-fastest).

Algorithm: the 4x4 sum only depends on the snapped point, so precompute
a 4x4 box-filtered map once in bf16, stored channel-last in DRAM as
boxfeat[row, c] with row = xl*512 + y*2 + xb (x = xb*128 + xl); each
point then becomes a single 512 B row-pair dma_gather (pair index
xl*256 + y <= 32766, int16).

Per core (2-way channel x 4-way ROI sharding):
  - filter: scalar-ACT converts feat chunks to bf16, DVE does the four
    separable shift-adds (SUM only; the /16 mean scale is applied
    host-side), one XBAR dma_start_transpose per y-chunk rearranges
    [c, (y,x)] -> [xl, (y,xb), c], contiguous 8 KiB-run DMA to DRAM.
  - 25600 pair indices computed on-device from the rois.
  - 8 dma_gather calls spread over 4 SWDGE queues fetch the row pairs;
    both x-halves go to out [25600, 256] bf16.
The host mirrors the f32 snap arithmetic (bit-exact) to pick the
correct x-half per point and reassembles the [20000, 1280] f32 output.
"""

import numpy as np

import concourse.bass as bass
import concourse.tile as tile
from concourse import bacc, mybir
from concourse import bass_utils

N_CORES = 8
CH_SHARD = 2          # channel shards (128 ch per core)
ROI_SHARD = 4         # ROI shards (5000 rois per core)
C, H, W = 256, 256, 256
CS = C // CH_SHARD    # 128 channels per core
N_ROIS = 20000
RPC = N_ROIS // ROI_SHARD          # 5000 rois per core
RP_PAD = 5120                       # padded to 40 rois per partition
RPP = RP_PAD // 128                 # 40 rois per partition
G = RPP * 5                         # 200 points per partition
NPTS_PAD = 128 * G                  # 25600 gathered points per core
YCHUNK = 16                         # output rows of the box filter per chunk
GCALLS = 8                          # gather calls
GN = NPTS_PAD // GCALLS             # 3200 points per gather call
GSL = GN // 128                     # 25 out slots per partition per call
F32 = mybir.dt.float32
BF16 = mybir.dt.bfloat16
I32 = mybir.dt.int32
I16 = mybir.dt.int16

_prog_cache = {}


def _build_program():
    nc = bacc.Bacc("TRN2", target_bir_lowering=False, debug=False,
                   num_devices=N_CORES, num_swdge_queues=4)

    feat_in = nc.dram_tensor("feat", [CS, H, W], F32, kind="ExternalInput")
    rois_in = nc.dram_tensor("rois", [RP_PAD, 8], F32, kind="ExternalInput")
    out_t = nc.dram_tensor("out", [NPTS_PAD, 2 * CS], BF16,
                           kind="ExternalOutput")
    # row = xl*512 + y*2 + xb  (x = xb*128 + xl), c contiguous
    boxfeat = nc.dram_tensor("boxfeat", [H * W, CS], BF16, kind="Internal")

    with tile.TileContext(nc) as tc:
        with (
            tc.tile_pool(name="sbuf", bufs=1) as pool,
            tc.tile_pool(name="io", bufs=2) as iop,
        ):
            idx16 = pool.tile([128, G], I16, tag="idx16")
            idx_w = []
            for c in range(GCALLS):
                w = pool.tile([128, G], I16, tag=f"idxw{c}")
                idx_w.append(w)

            # ---------------- point indices from rois -------------------
            if True:
                xp = pool
                roi_t = xp.tile([128, RPP, 8], F32, tag="roi")
                nc.sync.dma_start(
                    out=roi_t[:],
                    in_=rois_in.rearrange("(p r) c -> p r c", p=128),
                )
                rr = xp.tile([128, RPP, 8], F32, tag="rr")
                nc.vector.tensor_scalar_mul(rr[:], roi_t[:], 0.25)

                # points [128, RPP, 5] per coordinate, point k = slot k
                idx_f = {}
                for d in range(2):  # 0=x, 1=y
                    pts = xp.tile([128, RPP, 5], F32, tag=f"pts{d}")
                    # mids k=0..2: rr[2k+d] + rr[2k+2+d]
                    nc.vector.tensor_tensor(
                        out=pts[:, :, 0:3],
                        in0=rr[:, :, d:d + 5:2],
                        in1=rr[:, :, d + 2:d + 7:2],
                        op=mybir.AluOpType.add,
                    )
                    # mid k=3 wraps: rr[6+d] + rr[d]
                    nc.vector.tensor_tensor(
                        out=pts[:, :, 3:4],
                        in0=rr[:, :, d + 6:d + 7],
                        in1=rr[:, :, d:d + 1],
                        op=mybir.AluOpType.add,
                    )
                    nc.vector.tensor_scalar_mul(
                        pts[:, :, 0:4], pts[:, :, 0:4], 0.5)
                    # centroid, sequential sum order ((c0+c1)+c2)+c3
                    nc.vector.tensor_tensor(
                        out=pts[:, :, 4:5], in0=rr[:, :, d:d + 1],
                        in1=rr[:, :, d + 2:d + 3], op=mybir.AluOpType.add)
                    nc.vector.tensor_tensor(
                        out=pts[:, :, 4:5], in0=pts[:, :, 4:5],
                        in1=rr[:, :, d + 4:d + 5], op=mybir.AluOpType.add)
                    nc.vector.tensor_tensor(
                        out=pts[:, :, 4:5], in0=pts[:, :, 4:5],
                        in1=rr[:, :, d + 6:d + 7], op=mybir.AluOpType.add)
                    nc.vector.tensor_scalar_mul(
                        pts[:, :, 4:5], pts[:, :, 4:5], 0.25)

                    # ceil(x) = n + (x > n) where n = int-cast(x); works for
                    # either truncating or round-to-nearest casts.
                    ni = xp.tile([128, RPP, 5], I32, tag=f"ni{d}")
                    nc.vector.tensor_copy(out=ni[:], in_=pts[:])
                    tt = xp.tile([128, RPP, 5], F32, tag=f"tt{d}")
                    nc.vector.tensor_copy(out=tt[:], in_=ni[:])
                    gt = xp.tile([128, RPP, 5], F32, tag=f"gt{d}")
                    nc.vector.tensor_tensor(
                        out=gt[:], in0=pts[:], in1=tt[:],
                        op=mybir.AluOpType.is_gt)
                    nc.vector.tensor_tensor(
                        out=tt[:], in0=tt[:], in1=gt[:],
                        op=mybir.AluOpType.add)
                    # clip to [2, 254]
                    nc.vector.tensor_scalar(
                        out=tt[:], in0=tt[:], scalar1=2.0, scalar2=254.0,
                        op0=mybir.AluOpType.max, op1=mybir.AluOpType.min)
                    idx_f[d] = tt

                # pair index = xl*256 + y with xl = x - 128*(x >= 128);
                # max 127*256 + 254 = 32766, int16-safe.
                xcf, ycf = idx_f[0], idx_f[1]
                xbf = xp.tile([128, RPP, 5], F32, tag="xbf")
                nc.vector.tensor_scalar(
                    out=xbf[:], in0=xcf[:], scalar1=128.0, scalar2=None,
                    op0=mybir.AluOpType.is_ge)
                xlf = xp.tile([128, RPP, 5], F32, tag="xlf")
                nc.vector.scalar_tensor_tensor(
                    out=xlf[:], in0=xbf[:], scalar=-128.0, in1=xcf[:],
                    op0=mybir.AluOpType.mult, op1=mybir.AluOpType.add)
                flat_f = xp.tile([128, RPP, 5], F32, tag="flatf")
                nc.vector.scalar_tensor_tensor(
                    out=flat_f[:], in0=xlf[:], scalar=256.0, in1=ycf[:],
                    op0=mybir.AluOpType.mult, op1=mybir.AluOpType.add)
                nc.vector.tensor_copy(
                    out=idx16[:].rearrange("p (r k) -> p r k", k=5),
                    in_=flat_f[:])

                # dma_gather reads indices from partitions 0..15 (slot s,
                # part q -> stream position i = s*16+q), replicated to all
                # 8 groups of 16 partitions. Gather call c uses window
                # [16c, 16c+16): stream i of call c is point
                # (16c + i%16)*G + i//16; the host inverts that.
                engs = [nc.sync, nc.scalar, nc.sync, nc.scalar]
                for c in range(GCALLS):
                    for u in range(8):
                        engs[u % 4].dma_start(
                            out=idx_w[c][16 * u:16 * u + 16, :],
                            in_=idx16[16 * c:16 * c + 16, :])

            # boxfeat viewed [xl, y, xb, c]
            dst0 = boxfeat.rearrange("(xl y xb) c -> xl y xb c", xl=128, xb=2)

            # rows with y in {0,1,255} are never snapped to but are read
            # as pair halves: zero-fill so every gather read is finite.
            zt = pool.tile([128, 3, 2, CS], BF16, tag="zt")
            nc.vector.memset(zt[:], 0.0)
            nc.sync.dma_start(out=dst0[:, 0:2, :, :], in_=zt[:, 0:2])
            nc.sync.dma_start(out=dst0[:, 255:256, :, :], in_=zt[:, 2:3])

            # ---------------- box filter ---------------------------------
            # 4x4 box SUM (the /16 is applied host-side) with windows
            # [i-2, i+1] in both axes.
            if True:
                fp = pool
                for ci in range(H // YCHUNK):
                    a = max(2, ci * YCHUNK)            # first valid out row
                    b = min(H - 1, (ci + 1) * YCHUNK)  # end of valid rows
                    nv = b - a
                    ys0 = a - 2
                    ys1 = min(H, b + 1)                # u[y] needs h[y+1]
                    nr = ys1 - ys0                     # loaded rows (<= 19)

                    fin = iop.tile([128, YCHUNK + 3, W], F32, tag="fin")
                    nc.scalar.dma_start(
                        out=fin[:, 0:nr, :], in_=feat_in[:, ys0:ys1, :])
                    # bf16 convert on the otherwise-idle scalar ACT engine
                    # so every DVE add runs at the 16-bit rate
                    fb = iop.tile([128, YCHUNK + 3, W], BF16, tag="fb")
                    nc.scalar.activation(
                        out=fb[:, 0:nr, :], in_=fin[:, 0:nr, :],
                        func=mybir.ActivationFunctionType.Copy, scale=1.0)

                    s1 = fp.tile([128, YCHUNK + 3, W - 1], BF16, tag="s1")
                    nc.vector.tensor_tensor(
                        out=s1[:, 0:nr, :], in0=fb[:, 0:nr, 0:W - 1],
                        in1=fb[:, 0:nr, 1:W], op=mybir.AluOpType.add)
                    hh = fp.tile([128, YCHUNK + 3, W], BF16, tag="hh")
                    nc.vector.tensor_tensor(
                        out=hh[:, 0:nr, 2:W - 1], in0=s1[:, 0:nr, 0:W - 3],
                        in1=s1[:, 0:nr, 2:W - 1], op=mybir.AluOpType.add)
                    uu = fp.tile([128, YCHUNK + 2, W], BF16, tag="uu")
                    nc.vector.tensor_tensor(
                        out=uu[:, 0:nr - 1, 2:W - 1],
                        in0=hh[:, 0:nr - 1, 2:W - 1],
                        in1=hh[:, 1:nr, 2:W - 1], op=mybir.AluOpType.add)
                    # v[y'] = u[y'-2] + u[y']; double-buffered (consumed by
                    # the XBAR on another engine)
                    vv = iop.tile([128, YCHUNK, W], BF16, tag="vv")
                    nc.vector.memset(vv[:, :, 0:2], 0.0)
                    nc.vector.memset(vv[:, :, W - 1:W], 0.0)
                    o0 = a - 2 - ys0
                    o1 = a - ys0
                    nc.vector.tensor_tensor(
                        out=vv[:, 0:nv, 2:W - 1],
                        in0=uu[:, o0:o0 + nv, 2:W - 1],
                        in1=uu[:, o1:o1 + nv, 2:W - 1],
                        op=mybir.AluOpType.add)

                    # XBAR: stg[xl, y*2+xb, c] = vv[c, y, xb*128+xl]
                    stg = iop.tile([128, YCHUNK, 2, CS], BF16, tag="stg")
                    # keep every XBAR on ONE queue: concurrent transposes on
                    # both hwdge queues corrupt data (shared crossbar)
                    nc.sync.dma_start_transpose(
                        out=stg[:, 0:nv, :, :],
                        in_=vv[:, 0:nv, :],
                    )
                    # per-partition contiguous runs of nv*2*CS bf16,
                    # via the SWDGE mainline (queue 0)
                    nc.gpsimd.dma_start(
                        out=dst0[:, a:b, :, :],
                        in_=stg[:, 0:nv, :, :],
                    )

            # ---------------- gather + writeback -------------------------
            # DRAM row r = gi*GN + stream i; host un-permutes to point order
            out_v = out_t.rearrange("(s p) c -> p s c", p=128)
            pairs = boxfeat.rearrange("(r two) c -> r (two c)", two=2)
            for gi in range(GCALLS):
                gt = iop.tile([128, GSL, 2 * CS], BF16, tag="gt")
                nc.gpsimd.dma_gather(
                    gt[:],
                    pairs,
                    idx_w[gi][:],
                    GN,
                    GN,
                    2 * CS,
                    single_packet=False,
                    queue_num=gi % 4,
                )
                nc.sync.dma_start(
                    out=out_v[:, gi * GSL:(gi + 1) * GSL, :],
                    in_=gt[:])

    nc.compile()
    return nc


def _host_xhalf(rois: np.ndarray) -> np.ndarray:
    """Mirror the device f32 snap math; return per-point x-half (0/1).

    Bit-exact with the DVE ops: same f32 operand order for mids/centroid,
    ceil == int-snap + adjust == np.ceil for x >= 0, then clip [2, 254].
    """
    rr = (rois.astype(np.float32) * np.float32(0.25)).reshape(-1, 4, 2)
    mids = (rr + np.roll(rr, -1, axis=1)) * np.float32(0.5)
    ctr = ((rr[:, 0] + rr[:, 1]) + rr[:, 2]) + rr[:, 3]
    ctr = ctr * np.float32(0.25)
    px = np.concatenate([mids[:, :, 0], ctr[:, 0:1]], axis=1)  # [N, 5]
    xc = np.clip(np.ceil(px), np.float32(2.0), np.float32(254.0))
    return (xc >= 128.0).astype(np.int64)


def kernel(feat: np.ndarray, rois: np.ndarray) -> np.ndarray:
    feat = np.ascontiguousarray(np.asarray(feat, dtype=np.float32))
    rois = np.ascontiguousarray(np.asarray(rois, dtype=np.float32))
    assert feat.shape == (C, H, W) and rois.shape == (N_ROIS, 8)

    if "nc" not in _prog_cache:
        _prog_cache["nc"] = _build_program()
    nc = _prog_cache["nc"]

    rois_pad = np.zeros((RP_PAD, 8), dtype=np.float32)
    in_maps = []
    for core in range(N_CORES):
        ci, ri = divmod(core, ROI_SHARD)
        rp = rois_pad.copy()
        rp[:RPC] = rois[ri * RPC:(ri + 1) * RPC]
        in_maps.append({
            "feat": np.ascontiguousarray(feat[ci * CS:(ci + 1) * CS]),
            "rois": rp,
        })

    res = bass_utils.run_bass_kernel_spmd(
        nc, in_maps, core_ids=list(range(N_CORES)))

    # DRAM row r = c*GN + i holds point (16c + i%16)*G + i//16
    r = np.arange(NPTS_PAD)
    gc, i = divmod(r, GN)
    perm = (16 * gc + i % 16) * G + i // 16

    xhalf = _host_xhalf(rois)  # [N_ROIS, 5]
    out = np.empty((ROI_SHARD, RPC, 5, CH_SHARD, CS), dtype=np.float32)
    pts = np.empty((NPTS_PAD, 2 * CS), dtype=np.float32)
    for core in range(N_CORES):
        ci, ri = divmod(core, ROI_SHARD)
        # device values are unscaled 4x4 sums; apply the mean's 1/16 here
        pts[perm] = np.asarray(res.results[core]["out"]).astype(np.float32)
        pts *= np.float32(1.0 / 16.0)
        both = pts[:RPC * 5].reshape(RPC, 5, 2, CS)
        sel = np.take_along_axis(
            both,
            xhalf[ri * RPC:(ri + 1) * RPC][:, :, None, None],
            axis=2,
        )[:, :, 0]
        out[ri, :, :, ci, :] = sel
    return out.reshape(N_ROIS, 5 * C)


# revision 17
# speedup vs baseline: 1.7437x; 1.2020x over previous
"""NodeRoIPool Trainium2 kernel (v2).

For each of 20000 ROIs (8 corner coords), 5 points (4 edge midpoints +
centroid) are snapped to the feature grid (ceil, clip to [2,254]) and a
4x4 window of feat [256,256,256] is mean-pooled across all 256 channels,
giving out [20000, 1280] (point-major, channel-fastest).

Algorithm: the 4x4 mean only depends on the snapped point, so precompute
a 4x4 box-filtered feature map once (bf16, pre-scaled by 1/16), stored
channel-last in DRAM as boxfeat[row, c] with row = xl*512 + y*2 + xb
(x = xb*128 + xl); each point then becomes a single row-pair gather.

Per core (2-way channel x 4-way ROI sharding):
  - box filter: DVE shift-adds in bf16 (convert+1/16 scale on gpsimd),
    then ONE XBAR dma_start_transpose per y-chunk ([c, y, x] ->
    [xl, (y, xb), c]) and a 128-descriptor DMA to DRAM boxfeat.
  - 25600 pair indices (xl*256 + y <= 32766, int16) computed on-device.
  - 8 dma_gather calls spread over 4 SWDGE queues fetch 512B row pairs;
    both x-halves (xb=0/1) are written to out [25600, 256] bf16.
Host mirrors the f32 snap arithmetic to pick the correct x-half per
point (bit-exact with the device DVE ops) and reassembles the
[20000, 1280] f32 output.
"""

import numpy as np

import concourse.bass as bass
import concourse.tile as tile
from concourse import bacc, mybir
from concourse import bass_utils

N_CORES = 8
CH_SHARD = 2          # channel shards (128 ch per core)
ROI_SHARD = 4         # ROI shards (5000 rois per core)
C, H, W = 256, 256, 256
CS = C // CH_SHARD    # 128 channels per core
N_ROIS = 20000
RPC = N_ROIS // ROI_SHARD          # 5000 rois per core
RP_PAD = 5120                       # padded to 40 rois per partition
RPP = RP_PAD // 128                 # 40 rois per partition
G = RPP * 5                         # 200 points per partition
NPTS_PAD = 128 * G                  # 25600 rows in the padded output
YCHUNK = 16                         # output rows of the box filter per chunk
GCALLS = 8                          # gather calls
GN = NPTS_PAD // GCALLS             # 3200 points per gather call
GSL = GN // 128                     # 25 out slots per partition per call
F32 = mybir.dt.float32
BF16 = mybir.dt.bfloat16
I32 = mybir.dt.int32
I16 = mybir.dt.int16

_prog_cache = {}


def _build_program():
    nc = bacc.Bacc("TRN2", target_bir_lowering=False, debug=False,
                   num_devices=N_CORES, num_swdge_queues=4)

    feat_in = nc.dram_tensor("feat", [CS, H, W], F32, kind="ExternalInput")
    rois_in = nc.dram_tensor("rois", [RP_PAD, 8], F32, kind="ExternalInput")
    out_t = nc.dram_tensor("out", [NPTS_PAD, 2 * CS], BF16,
                           kind="ExternalOutput")
    # row = xl*512 + y*2 + xb  (x = xb*128 + xl), c contiguous
    boxfeat = nc.dram_tensor("boxfeat", [H * W, CS], BF16, kind="Internal")

    with tile.TileContext(nc) as tc:
        with (
            tc.tile_pool(name="sbuf", bufs=1) as pool,
            tc.tile_pool(name="io", bufs=2) as iop,
        ):
            # ---------------- point indices from rois -------------------
            # rois tile: partition p holds rois [p*40, (p+1)*40)
            roi_t = pool.tile([128, RPP, 8], F32, tag="roi")
            nc.sync.dma_start(
                out=roi_t[:],
                in_=rois_in.rearrange("(p r) c -> p r c", p=128),
            )
            rr = pool.tile([128, RPP, 8], F32, tag="rr")
            nc.vector.tensor_scalar_mul(rr[:], roi_t[:], 0.25)

            # points [128, RPP, 5] per coordinate, point k = slot k
            idx_f = {}
            for d in range(2):  # 0=x, 1=y
                pts = pool.tile([128, RPP, 5], F32, tag=f"pts{d}")
                # mids k=0..2: rr[2k+d] + rr[2k+2+d]
                nc.vector.tensor_tensor(
                    out=pts[:, :, 0:3],
                    in0=rr[:, :, d:d + 5:2],
                    in1=rr[:, :, d + 2:d + 7:2],
                    op=mybir.AluOpType.add,
                )
                # mid k=3 wraps: rr[6+d] + rr[d]
                nc.vector.tensor_tensor(
                    out=pts[:, :, 3:4],
                    in0=rr[:, :, d + 6:d + 7],
                    in1=rr[:, :, d:d + 1],
                    op=mybir.AluOpType.add,
                )
                nc.vector.tensor_scalar_mul(pts[:, :, 0:4], pts[:, :, 0:4], 0.5)
                # centroid, sequential sum order ((c0+c1)+c2)+c3
                nc.vector.tensor_tensor(
                    out=pts[:, :, 4:5], in0=rr[:, :, d:d + 1],
                    in1=rr[:, :, d + 2:d + 3], op=mybir.AluOpType.add)
                nc.vector.tensor_tensor(
                    out=pts[:, :, 4:5], in0=pts[:, :, 4:5],
                    in1=rr[:, :, d + 4:d + 5], op=mybir.AluOpType.add)
                nc.vector.tensor_tensor(
                    out=pts[:, :, 4:5], in0=pts[:, :, 4:5],
                    in1=rr[:, :, d + 6:d + 7], op=mybir.AluOpType.add)
                nc.vector.tensor_scalar_mul(pts[:, :, 4:5], pts[:, :, 4:5], 0.25)

                # ceil(x) = n + (x > n) where n = int-cast(x); works for
                # either truncating or round-to-nearest casts since
                # n in {floor, ceil} and |n - x| < 1 for x >= 0.
                ni = pool.tile([128, RPP, 5], I32, tag=f"ni{d}")
                nc.vector.tensor_copy(out=ni[:], in_=pts[:])
                tt = pool.tile([128, RPP, 5], F32, tag=f"tt{d}")
                nc.vector.tensor_copy(out=tt[:], in_=ni[:])
                gt = pool.tile([128, RPP, 5], F32, tag=f"gt{d}")
                nc.vector.tensor_tensor(
                    out=gt[:], in0=pts[:], in1=tt[:], op=mybir.AluOpType.is_gt)
                nc.vector.tensor_tensor(
                    out=tt[:], in0=tt[:], in1=gt[:], op=mybir.AluOpType.add)
                # clip to [2, 254]
                nc.vector.tensor_scalar(
                    out=tt[:], in0=tt[:], scalar1=2.0, scalar2=254.0,
                    op0=mybir.AluOpType.max, op1=mybir.AluOpType.min)
                idx_f[d] = tt

            # pair index = xl*256 + y with xl = x - 128*(x >= 128).
            # Max 127*256 + 254 = 32766, fits int16.
            xcf, ycf = idx_f[0], idx_f[1]
            xbf = pool.tile([128, RPP, 5], F32, tag="xbf")
            nc.vector.tensor_scalar(
                out=xbf[:], in0=xcf[:], scalar1=128.0, scalar2=None,
                op0=mybir.AluOpType.is_ge)
            # xl = xb * (-128) + x
            xlf = pool.tile([128, RPP, 5], F32, tag="xlf")
            nc.vector.scalar_tensor_tensor(
                out=xlf[:], in0=xbf[:], scalar=-128.0, in1=xcf[:],
                op0=mybir.AluOpType.mult, op1=mybir.AluOpType.add)
            # flat = xl * 256 + y
            flat_f = pool.tile([128, RPP, 5], F32, tag="flatf")
            nc.vector.scalar_tensor_tensor(
                out=flat_f[:], in0=xlf[:], scalar=256.0, in1=ycf[:],
                op0=mybir.AluOpType.mult, op1=mybir.AluOpType.add)
            idx16 = pool.tile([128, G], I16, tag="idx16")
            nc.vector.tensor_copy(
                out=idx16[:].rearrange("p (r k) -> p r k", k=5), in_=flat_f[:])

            # dma_gather reads indices from partitions 0..15 (slot s, part q
            # -> stream position i = s*16+q), replicated to all 8 groups of
            # 16 partitions, and emits stream position i at out[i%128,
            # i//128]. Gather call c uses the computed tile's partition
            # window [16c, 16c+16): stream i of call c is point
            # (16c + i%16)*G + i//16, and the HOST inverts that fixed
            # permutation for free.
            engs = [nc.sync, nc.scalar, nc.sync, nc.scalar]
            idx_w = []
            for c in range(GCALLS):
                w = pool.tile([128, G], I16, tag=f"idxw{c}")
                for u in range(8):
                    engs[u % 4].dma_start(
                        out=w[16 * u:16 * u + 16, :],
                        in_=idx16[16 * c:16 * c + 16, :])
                idx_w.append(w)

            # ---------------- box filter ---------------------------------
            # 4x4 box mean with windows [i-2, i+1] in both axes; outputs
            # only y',x' in [2, 254] are ever gathered.
            # boxfeat viewed [xl, y, xb, c]
            dst0 = boxfeat.rearrange("(xl y xb) c -> xl y xb c", xl=128, xb=2)

            # rows with y in {0,1,255} and cols x in {0,1,255} are never
            # snapped to, but pair gathers read both halves of a pair:
            # zero-fill so every gather read is finite.
            zt = pool.tile([128, 3, 2, CS], BF16, tag="zt")
            nc.vector.memset(zt[:], 0.0)
            nc.sync.dma_start(out=dst0[:, 0:2, :, :], in_=zt[:, 0:2])
            nc.sync.dma_start(out=dst0[:, 255:256, :, :], in_=zt[:, 2:3])

            for ci in range(H // YCHUNK):
                a = max(2, ci * YCHUNK)              # first valid out row
                b = min(H - 1, (ci + 1) * YCHUNK)    # end of valid out rows
                nv = b - a
                ys0 = a - 2
                ys1 = min(H, b + 1)                  # u[y] needs h[y+1]
                nr = ys1 - ys0                       # loaded rows (<= 19)

                fin = iop.tile([128, YCHUNK + 3, W], F32, tag="fin")
                # alternate hwdge queues for the big feat loads
                (nc.scalar if ci % 2 else nc.sync).dma_start(
                    out=fin[:, 0:nr, :], in_=feat_in[:, ys0:ys1, :])

                # first add reads f32 directly (f32 DVE rate), emits bf16;
                # the pooling 1/16 scale is applied host-side for free
                s1 = pool.tile([128, YCHUNK + 3, W - 1], BF16, tag="s1")
                nc.vector.tensor_tensor(
                    out=s1[:, 0:nr, :], in0=fin[:, 0:nr, 0:W - 1],
                    in1=fin[:, 0:nr, 1:W], op=mybir.AluOpType.add)
                hh = pool.tile([128, YCHUNK + 3, W], BF16, tag="hh")
                nc.vector.tensor_tensor(
                    out=hh[:, 0:nr, 2:W - 1], in0=s1[:, 0:nr, 0:W - 3],
                    in1=s1[:, 0:nr, 2:W - 1], op=mybir.AluOpType.add)
                uu = pool.tile([128, YCHUNK + 2, W], BF16, tag="uu")
                nc.vector.tensor_tensor(
                    out=uu[:, 0:nr - 1, 2:W - 1], in0=hh[:, 0:nr - 1, 2:W - 1],
                    in1=hh[:, 1:nr, 2:W - 1], op=mybir.AluOpType.add)
                vv = pool.tile([128, YCHUNK, W], BF16, tag="vv")
                # cols 0,1,255 are never snapped to but are transposed into
                # boxfeat (pair halves); zero them so gathers stay finite.
                nc.vector.memset(vv[:, :, 0:2], 0.0)
                nc.vector.memset(vv[:, :, W - 1:W], 0.0)
                # v[y'] = u[y'-2] + u[y']
                o0 = a - 2 - ys0
                o1 = a - ys0
                nc.vector.tensor_tensor(
                    out=vv[:, 0:nv, 2:W - 1],
                    in0=uu[:, o0:o0 + nv, 2:W - 1],
                    in1=uu[:, o1:o1 + nv, 2:W - 1],
                    op=mybir.AluOpType.add)

                # XBAR transpose [c, (y, x)] -> [xl, (y, xb), c] in one
                # instruction: out[xl, y*2+xb, c] = vv[c, y, xb*128+xl]
                stg = iop.tile([128, YCHUNK, 2, CS], BF16, tag="stg")
                (nc.sync if ci % 2 else nc.scalar).dma_start_transpose(
                    out=stg[:, 0:nv, :, :],
                    in_=vv[:, 0:nv, :],
                )
                # per-partition contiguous runs of nv*2*CS bf16; routed
                # through the (phase-1-idle) SWDGE mainline queue
                nc.gpsimd.dma_start(
                    out=dst0[:, a:b, :, :],
                    in_=stg[:, 0:nv, :, :],
                )

            # ---------------- gather + writeback -------------------------
            # DRAM row r = gi*GN + stream i; host un-permutes to point order
            out_v = out_t.rearrange("(s p) c -> p s c", p=128)
            pairs = boxfeat.rearrange("(r two) c -> r (two c)", two=2)
            for gi in range(GCALLS):
                gt = iop.tile([128, GSL, 2 * CS], BF16, tag="gather")
                nc.gpsimd.dma_gather(
                    gt[:],
                    pairs,
                    idx_w[gi][:],
                    GN,
                    GN,
                    2 * CS,
                    single_packet=False,
                    queue_num=gi % 4,
                )
                nc.sync.dma_start(
                    out=out_v[:, gi * GSL:(gi + 1) * GSL, :],
                    in_=gt[:])

    nc.compile()
    return nc


def _host_xhalf(rois: np.ndarray) -> np.ndarray:
    """Mirror the device f32 snap math; return per-point x-half (0/1).

    Bit-exact with the DVE ops: same f32 operand order for mids/centroid,
    ceil == int-snap + adjust == np.ceil for x >= 0, then clip [2, 254].
    """
    rr = (rois.astype(np.float32) * np.float32(0.25)).reshape(-1, 4, 2)
    mids = (rr + np.roll(rr, -1, axis=1)) * np.float32(0.5)
    ctr = ((rr[:, 0] + rr[:, 1]) + rr[:, 2]) + rr[:, 3]
    ctr = ctr * np.float32(0.25)
    px = np.concatenate([mids[:, :, 0], ctr[:, 0:1]], axis=1)  # [N, 5]
    xc = np.clip(np.ceil(px), np.float32(2.0), np.float32(254.0))
    return (xc >= 128.0).astype(np.int64)


def kernel(feat: np.ndarray, rois: np.ndarray) -> np.ndarray:
    feat = np.ascontiguousarray(np.asarray(feat, dtype=np.float32))
    rois = np.ascontiguousarray(np.asarray(rois, dtype=np.float32))
    assert feat.shape == (C, H, W) and rois.shape == (N_ROIS, 8)

    if "nc" not in _prog_cache:
        _prog_cache["nc"] = _build_program()
    nc = _prog_cache["nc"]

    rois_pad = np.zeros((RP_PAD, 8), dtype=np.float32)
    in_maps = []
    for core in range(N_CORES):
        ci, ri = divmod(core, ROI_SHARD)
        rp = rois_pad.copy()
        rp[:RPC] = rois[ri * RPC:(ri + 1) * RPC]
        in_maps.append({
            "feat": np.ascontiguousarray(feat[ci * CS:(ci + 1) * CS]),
            "rois": rp,
        })

    res = bass_utils.run_bass_kernel_spmd(
        nc, in_maps, core_ids=list(range(N_CORES)))

    # DRAM row r = c*GN + i holds point (16c + i%16)*G + i//16
    r = np.arange(NPTS_PAD)
    gc, i = divmod(r, GN)
    perm = (16 * gc + i % 16) * G + i // 16

    xhalf = _host_xhalf(rois)  # [N_ROIS, 5]
    out = np.empty((ROI_SHARD, RPC, 5, CH_SHARD, CS), dtype=np.float32)
    pts = np.empty((NPTS_PAD, 2 * CS), dtype=np.float32)
    for core in range(N_CORES):
        ci, ri = divmod(core, ROI_SHARD)
        # device sums are unscaled; the 4x4-mean 1/16 is applied here
        pts[perm] = np.asarray(res.results[core]["out"]).astype(np.float32)
        pts *= np.float32(1.0 / 16.0)
        both = pts[:RPC * 5].reshape(RPC, 5, 2, CS)
        sel = np.take_along_axis(
            both,
            xhalf[ri * RPC:(ri + 1) * RPC][:, :, None, None],
            axis=2,
        )[:, :, 0]
        out[ri, :, :, ci, :] = sel
    return out.reshape(N_ROIS, 5 * C)
